# revision 52
# baseline (speedup 1.0000x reference)
"""MoE routed matmul on 8 NeuronCores (Trainium2, Bass).

Problem: out[b] = x[b] @ W[idx[b]]  with  x:(2048,256), W:(64,256,256),
idx:(2048,1) int32.

Strategy: expert-parallel, bf16, blob-packed DMA. Experts (contexts)
are sharded 8-per-core. The host routes samples to the core that owns
their expert (the all-to-all, done during input sharding), padding each
expert's group to capacity CAP=64 so the device program is fully
static. Each core does 8 dense (64 x 256) @ (256 x 256) matmuls;
weights are read from HBM exactly once across the device, which is
what the memory-bound roofline wants.

The problem is memory-bound, so the design minimizes and streamlines
HBM traffic (vs a straight f32 port, ~2.2x):
  * bf16 for x, W and the output — halves HBM traffic. Tolerance is
    2e-2; bf16 end-to-end lands at ~3e-3.
  * The host packs x^T and all 8 expert weights into ONE contiguous
    DRAM blob per core [128 partitions x 10 KiB], so DMA lines are
    multi-KB and transfers run near the practical DMA ceiling. The
    device pulls it in 2 chunks (xt+w0..w4 | w5..w7): big enough to be
    efficient, split so the first 5 experts' matmuls overlap the
    second chunk's DMA.
  * Outputs are staged through one contiguous SBUF tile (PSUM->SBUF
    copies on the vector engine, casting f32->bf16) and written back
    in 2 x 128KB DMAs with 1KB lines; the copies and first write-back
    overlap the matmul tail.

Device program per core (raw Bass, manual semaphores):
  sync   : 2 chunked blob DMAs into SBUF (dedicated sem per chunk)
  tensor : per expert, 2 accumulating matmuls (K=256 split in 2) into a
           PSUM bank; expert pairs share a bank at partition offsets
           0/64 so downstream copies/DMAs are full 128-wide
  vector : PSUM -> SBUF copy per expert pair (f32 -> bf16)
  scalar : write-back DMA after every 2nd copy

For benchmarking, nloop > 1 wraps the body in an on-device Fori
hardware loop with iterations chained serially (each iteration's input
DMAs wait on the previous iteration's output DMAs), so one dispatch
measures nloop cold-call-equivalent executions back to back.
Semaphores use constant thresholds with a waiter-clears protocol: the
unique waiting engine clears each sem right after its wait passes,
race-free because every producer's next increment is transitively
gated behind that clear via the serial chain.
"""

import numpy as np
import ml_dtypes
from contextlib import ExitStack

B, D, U, C = 2048, 256, 256, 64
NCORES = 8
EPC = C // NCORES   # experts per core
CAP = 64            # per-expert sample capacity (padded)
NPAIR = EPC // 2
BF16 = ml_dtypes.bfloat16

_prog_cache: dict = {}


def _cols(cap: int):
    """Blob column layout: [xt (2*EPC*cap) | w (EPC*2*U)]."""
    xt_cols = 2 * EPC * cap
    return xt_cols, xt_cols + EPC * 2 * U


def _default_groups(cap: int):
    xt_cols, ncols = _cols(cap)
    # 4 chunks, issued alternating between the SP and ACT HWDGE rings
    # (dual): [xt,w0..w2 | w3,w4 | w5,+half w6 | rest]
    return (xt_cols + 3 * 2 * U, xt_cols + 5 * 2 * U, 4352, ncols)


def _build_loop_program(cap: int, nloop: int, groups=None, safe: bool = True,
                        vcopy: bool = True, osplit: int = 2, dual: bool = True):
    # safe=False drops the copy->write-back sem edge: WRONG on a fresh
    # executable's first run (timing experiments only)
    # vcopy: PSUM->SBUF copies on the vector engine with a cross-engine
    # sem edge to the scalar engine's write-backs
    import concourse.bass as bass
    from concourse import mybir
    from concourse.bass import compact_to_ranges

    f32 = mybir.dt.float32
    bf16 = mybir.dt.bfloat16
    assert cap % 2 == 0 and 2 * cap <= 128
    xt_cols, ncols = _cols(cap)
    if groups is None:
        groups = _default_groups(cap)
    groups = tuple(groups)
    assert groups[-1] == ncols and groups[0] >= xt_cols + 2 * U
    ngrp = len(groups)
    # output write-back chunking: int N = N equal chunks, or a tuple of
    # per-chunk pair counts (e.g. (3, 1) = big early chunk + small tail)
    splits = (
        tuple([NPAIR // osplit] * osplit) if isinstance(osplit, int) else tuple(osplit)
    )
    assert sum(splits) == NPAIR
    nsplit = len(splits)
    bounds = [0]
    for sp in splits:
        bounds.append(bounds[-1] + sp)

    def req(j):
        """First group index whose prefix covers xt + expert j's weights."""
        need = xt_cols + (j + 1) * 2 * U
        for g, b in enumerate(groups):
            if b >= need:
                return g
        raise AssertionError

    nc = bass.Bass()
    blob = nc.declare_dram_parameter("blob", [128, ncols], bf16, isOutput=False)
    out = nc.declare_dram_parameter("out", [2 * cap, NPAIR * U], bf16, isOutput=True)

    with ExitStack() as ctx:
        sb = ctx.enter_context(nc.sbuf_tensor("sb", [128, ncols], bf16))
        sb_all = ctx.enter_context(nc.sbuf_tensor("sb_all", [128, NPAIR * U], bf16))
        ps = [
            ctx.enter_context(nc.psum_tensor(f"ps{p}", [128, 512], f32))
            for p in range(NPAIR)
        ]
        # Dedicated sems per DMA: a wait threshold on a sem that counts
        # several in-flight DMAs is unsound (a DMA's +16 completion is
        # split +1 across 16 SDMA engines, so a later DMA's increments
        # can satisfy an earlier DMA's threshold while it still has a
        # straggler engine). One sem per DMA makes thresholds exact.
        g_sem = [ctx.enter_context(nc.semaphore(f"g_sem{g}")) for g in range(ngrp)]
        mm_sem = ctx.enter_context(nc.semaphore("mm_sem"))
        cp_sem = ctx.enter_context(nc.semaphore("cp_sem"))
        out_sem = [ctx.enter_context(nc.semaphore(f"out_sem{s}")) for s in range(nsplit)]
        go_sem = ctx.enter_context(nc.semaphore("go_sem")) if dual else None

        # Semaphores are NOT cleared when a loaded NEFF is re-executed, so
        # stale values would break waits on the second run. Clear the
        # whole kernel sem range up front, then a pseudo-sync barrier
        # keeps every engine parked until the clears retire.
        for sem_range in compact_to_ranges(
            [s for s in nc._kernel_sem_range if s not in nc.barrier_sems]
        ):
            nc.gpsimd.dma_reset(sem_range)
            nc.gpsimd.sem_clear(sem_range)
        nc._nrt_pseudo_barrier()
        # prime the serial chain so iteration 0's out_sem waits pass
        for s in range(nsplit):
            nc.gpsimd.sem_inc(out_sem[s], 16)

        ET = mybir.EngineType
        loop_engines = [ET.SP, ET.PE, ET.Activation] + ([ET.DVE] if vcopy else [])
        with nc.Fori(0, nloop, engines=loop_engines):
            # sync: wait for the previous iteration's write-backs, re-arm,
            # then pull this iteration's inputs
            for s in range(nsplit):
                nc.sync.wait_ge(out_sem[s], 16)
            for s in range(nsplit):
                nc.sync.sem_clear(out_sem[s])
            if dual:
                # release the scalar ring only once the serial chain allows
                # this iteration to begin (keeps strict serialization)
                nc.sync.sem_inc(go_sem, 1)
                nc.scalar.wait_ge(go_sem, 1)
                nc.scalar.sem_clear(go_sem)
            lo = 0
            for g in range(ngrp):
                hi = groups[g]
                # dual: odd chunks issue from the scalar engine's HWDGE
                # ring (qActDynamicHW), concurrent with the sync ring
                eng = nc.scalar if (dual and g % 2 == 1) else nc.sync
                eng.dma_start(sb[:, lo:hi], blob[:, lo:hi]).then_inc(g_sem[g], 16)
                lo = hi
            # tensor: 2 accumulating matmuls per expert, overlapping the
            # later input chunks
            prev_req = -1
            for j in range(EPC):
                p, half = j // 2, j % 2
                r = req(j)
                for g in range(prev_req + 1, r + 1):
                    nc.tensor.wait_ge(g_sem[g], 16)
                    nc.tensor.sem_clear(g_sem[g])
                prev_req = r
                for k in range(2):
                    mm = nc.tensor.matmul(
                        ps[p][half * cap:(half + 1) * cap, 0:U],
                        sb[:, k * EPC * cap + j * cap:
                           k * EPC * cap + (j + 1) * cap],
                        sb[:, xt_cols + (j * 2 + k) * U:
                           xt_cols + (j * 2 + k + 1) * U],
                        start=(k == 0),
                        stop=(k == 1),
                    )
                mm.then_inc(mm_sem, 1)
            # copies: each finished pair lands in the contiguous staging
            # tile (casting f32->bf16); write-back after every `per`-th
            # pair. The write-back DMA must wait on the copies' sem —
            # engine-order alone does NOT make copied data visible to the
            # SDMA engines before the doorbell (first-execution garbage
            # escapes without it).
            if vcopy:
                for p in range(NPAIR):
                    nc.vector.wait_ge(mm_sem, 2 * p + 2)
                    if p == NPAIR - 1:
                        nc.vector.sem_clear(mm_sem)
                    nc.vector.tensor_copy(
                        sb_all[:, p * U:(p + 1) * U], ps[p][:, 0:U]
                    ).then_inc(cp_sem, 1)
                for s in range(nsplit):
                    lo, hi = bounds[s] * U, bounds[s + 1] * U
                    nc.scalar.wait_ge(cp_sem, bounds[s + 1])
                    if s == nsplit - 1:
                        nc.scalar.sem_clear(cp_sem)
                    nc.scalar.dma_start(
                        out[:, lo:hi], sb_all[0:2 * cap, lo:hi]
                    ).then_inc(out_sem[s], 16)
            else:
                for p in range(NPAIR):
                    nc.scalar.wait_ge(mm_sem, 2 * p + 2)
                    if p == NPAIR - 1:
                        nc.scalar.sem_clear(mm_sem)
                    nc.scalar.copy(
                        sb_all[:, p * U:(p + 1) * U], ps[p][:, 0:U]
                    ).then_inc(cp_sem, 1)
                    if (p + 1) in bounds[1:]:
                        s = bounds.index(p + 1) - 1
                        lo, hi = bounds[s] * U, bounds[s + 1] * U
                        if safe:
                            nc.scalar.wait_ge(cp_sem, bounds[s + 1])
                            if s == nsplit - 1:
                                nc.scalar.sem_clear(cp_sem)
                        nc.scalar.dma_start(
                            out[:, lo:hi], sb_all[0:2 * cap, lo:hi]
                        ).then_inc(out_sem[s], 16)
        # quiesce: the final iteration's write-backs must land before halt
        for s in range(nsplit):
            nc.sync.wait_ge(out_sem[s], 16)

    return nc


def _route(content_idx: np.ndarray, x: np.ndarray, cap: int):
    """Sort samples by expert; compute per-core packed bf16 x^T shards."""
    idx = content_idx.reshape(-1).astype(np.int64)
    order = np.argsort(idx, kind="stable")
    e_sorted = idx[order]
    counts = np.bincount(idx, minlength=C)
    while counts.max() > cap:
        cap *= 2
    start = np.zeros(C, dtype=np.int64)
    start[1:] = np.cumsum(counts)[:-1]
    slot = np.arange(B) - start[e_sorted]
    core = e_sorted // EPC
    col = (e_sorted % EPC) * cap + slot

    xs = np.zeros((NCORES, EPC * cap, D), dtype=np.float32)
    xs[core, col] = x[order]
    # [c, rows, D] -> [c, 128, 2*rows] with col index k*rows + row,
    # matching the device's xt slice k*EPC*cap + j*cap + slot
    xt_all = (
        xs.transpose(0, 2, 1)
        .reshape(NCORES, 2, 128, EPC * cap)
        .transpose(0, 2, 1, 3)
        .reshape(NCORES, 128, 2 * EPC * cap)
        .astype(BF16)
    )
    return cap, order, core, col, xt_all


def _unshard(outs: np.ndarray, order, core, col, cap: int) -> np.ndarray:
    """Scatter per-core padded device output back to original sample order."""
    a = (
        outs.astype(np.float32)
        .reshape(NCORES, 2 * cap, NPAIR, U)
        .transpose(0, 2, 1, 3)
        .reshape(NCORES, EPC * cap, U)
    )
    out_full = np.empty((B, U), dtype=np.float32)
    out_full[order] = a[core, col]
    return out_full


def _make_in_maps(xt_all: np.ndarray, kernel_w: np.ndarray):
    # kernel [C, D, U] -> per-core [128, EPC*2*U], partition-major so the
    # blob DMA reads multi-KB contiguous lines per partition
    w = (
        np.ascontiguousarray(kernel_w, dtype=np.float32)
        .reshape(NCORES, EPC, 2, 128, U)
        .transpose(0, 3, 1, 2, 4)
        .reshape(NCORES, 128, EPC * 2 * U)
        .astype(BF16)
    )
    blob = np.concatenate([np.ascontiguousarray(xt_all), w], axis=2)
    return [{"blob": np.ascontiguousarray(blob[c])} for c in range(NCORES)]


def kernel(content_idx: np.ndarray, x: np.ndarray, kernel: np.ndarray) -> np.ndarray:
    from concourse.bass_utils import run_bass_kernel_spmd

    cap, order, core, col, xt_all = _route(content_idx, x, CAP)
    if cap > CAP:
        # Pathologically skewed routing (an expert holds >CAP samples) can't
        # use the static pair-packed program. Unreachable for the fixed-seed
        # problem data; fall back to a host computation to stay correct.
        idx = content_idx.reshape(-1).astype(np.int64)
        return np.einsum("bd,bdu->bu", x.astype(np.float32),
                         kernel.astype(np.float32)[idx]).astype(np.float32)

    if cap not in _prog_cache:
        _prog_cache[cap] = _build_loop_program(cap, 1)
    nc = _prog_cache[cap]

    in_maps = _make_in_maps(xt_all, kernel)
    res = run_bass_kernel_spmd(nc, in_maps, list(range(NCORES)))
    outs = np.stack([res.results[c]["out"] for c in range(NCORES)])
    return _unshard(outs, order, core, col, cap)
